# revision 4
# baseline (speedup 1.0000x reference)
"""Trainium2 Bass kernel for nn_BilinAndFwdComboVecComp.

Math (B=8, S=256, C=256, V=64):
  final[b,s,z,k] = tanh( sum_ij ctx[b,s,i] ctx[b,z,j] W'[i,j,k] + A[b,z,k] + Bt[b,s,k] )
where
  W'[i,j,k] = W[i,j,k] + (i==j) * linmul_w[k,i]          (folds the `mul` branch)
  A[b,z,k]  = ctx[b] @ (lin1_w+lindiff_w).T + (lin1_b + bias + linmul_b + lindiff_b)
  Bt[b,s,k] = ctx[b] @ (lin2_w-lindiff_w).T + lin2_b     (the `diff` branch is rank-1
                                                          per pair and merges into A/Bt)

Sharding: V split across the 8 cores (8 k-values per core). Each core:
  phase 1: tmp2_b[i,(k,z)] = sum_j Wt[j,(k,i)]^T-slices @ ctxT_b[j,z]   (W-stationary)
  phase 2: out[s,(z,k)]    = ctxT_b[:,s]^T @ tmp2_b[:,(z,k)]  (+ K=9 fold matmul that
           adds A via a ones-row and Bt via delta-rows), then tanh on ACT, DMA out.
All matmuls run as float32r (full PE rate at moving-dim >= 256, fp32-class precision).
Per-core output scratch (B,S,S,8) is concatenated on the host along k.
"""

import numpy as np

B, S, C, V = 8, 256, 256, 64
NCORES = 8
KV = V // NCORES  # k-values per core


def _host_prep(ctx, W, bias, lin1_w, lin1_b, lin2_w, lin2_b,
               linmul_w, linmul_b, lindiff_w, lindiff_b):
    f = np.float32
    ctx = np.asarray(ctx, f)
    Wp = np.array(W, f)
    Wp[np.arange(C), np.arange(C), :] += np.asarray(linmul_w, f).T
    Wt = Wp.transpose(1, 0, 2)  # [j, i, k]

    A = ctx @ (np.asarray(lin1_w, f) + np.asarray(lindiff_w, f)).T \
        + (np.asarray(lin1_b, f) + np.asarray(bias, f)
           + np.asarray(linmul_b, f) + np.asarray(lindiff_b, f))
    Bt = ctx @ (np.asarray(lin2_w, f) - np.asarray(lindiff_w, f)).T + np.asarray(lin2_b, f)

    ctxT = np.ascontiguousarray(ctx.transpose(0, 2, 1))  # [B, C, S]

    delta = np.zeros((KV, S * KV), f)
    for r in range(KV):
        delta[r, r::KV] = 1.0

    per_core = []
    for c in range(NCORES):
        ks = slice(c * KV, (c + 1) * KV)
        # wt layout: [j, kk*C + i]
        wt = np.ascontiguousarray(Wt[:, :, ks].transpose(0, 2, 1).reshape(C, KV * C))
        # foldL: [9, b*S + s]; rows 0..KV-1 = Bt slice, row KV = ones
        foldL = np.empty((KV + 1, B * S), f)
        foldL[:KV] = Bt[:, :, ks].transpose(2, 0, 1).reshape(KV, B * S)
        foldL[KV] = 1.0
        # foldR: [b, 9, z*KV + kk]; rows 0..KV-1 = delta, row KV = A slice
        foldR = np.empty((B, KV + 1, S * KV), f)
        foldR[:, :KV, :] = delta[None]
        foldR[:, KV, :] = A[:, :, ks].reshape(B, S * KV)
        per_core.append({"ctxT": ctxT, "wt": wt, "foldL": foldL, "foldR": foldR})
    return per_core


def _build_program():
    import concourse.tile as tile
    import concourse.mybir as mybir
    from concourse import bacc
    from contextlib import ExitStack

    f32 = mybir.dt.float32
    f32r = mybir.dt.float32r
    TANH = mybir.ActivationFunctionType.Tanh

    nc = bacc.Bacc("TRN2", target_bir_lowering=False, debug=False)
    ctxT_d = nc.dram_tensor("ctxT", [B, C, S], f32r, kind="ExternalInput").ap()
    wt_d = nc.dram_tensor("wt", [C, KV * C], f32r, kind="ExternalInput").ap()
    foldL_d = nc.dram_tensor("foldL", [KV + 1, B * S], f32r, kind="ExternalInput").ap()
    foldR_d = nc.dram_tensor("foldR", [B, KV + 1, S * KV], f32r, kind="ExternalInput").ap()
    out_d = nc.dram_tensor("out", [B, S, S, KV], f32, kind="ExternalOutput").ap()

    with tile.TileContext(nc) as tc, ExitStack() as es:
        ctx_pool = es.enter_context(tc.tile_pool(name="ctxp", bufs=16))
        wt_pool = es.enter_context(tc.tile_pool(name="wtp", bufs=2))
        fl_pool = es.enter_context(tc.tile_pool(name="flp", bufs=1))
        fr_pool = es.enter_context(tc.tile_pool(name="frp", bufs=2))
        tmp2_pool = es.enter_context(tc.tile_pool(name="tmp2p", bufs=12))
        outs_pool = es.enter_context(tc.tile_pool(name="outsp", bufs=3))
        ps1_pool = es.enter_context(tc.tile_pool(name="ps1", bufs=4, space="PSUM"))
        ps2_pool = es.enter_context(tc.tile_pool(name="ps2", bufs=4, space="PSUM"))

        ctx_sb = {}
        for b in range(B):
            for ch in range(2):
                t = ctx_pool.tile([128, S], f32r, name=f"ctx_{b}_{ch}", bufs=1)
                nc.sync.dma_start(t[:], ctxT_d[b, ch * 128:(ch + 1) * 128, :])
                ctx_sb[b, ch] = t
        wt_sb = []
        for j in range(2):
            t = wt_pool.tile([128, KV * C], f32r, name=f"wt_{j}", bufs=1)
            nc.sync.dma_start(t[:], wt_d[j * 128:(j + 1) * 128, :])
            wt_sb.append(t)
        foldL_sb = fl_pool.tile([KV + 1, B * S], f32r, name="foldL", bufs=1)
        nc.sync.dma_start(foldL_sb[:], foldL_d[:])

        tmp2 = {}

        def phase1(bg):
            for ch in range(2):  # i-chunk (output partition of tmp2)
                for b in bg:
                    tmp2[b, ch] = tmp2_pool.tile([128, KV * S], f32r, name="tmp2")
                for kk in range(KV):
                    ps = {}
                    for b in bg:
                        ps[b] = ps1_pool.tile([128, S], f32, name="ps1")
                    for j in range(2):  # contraction chunk
                        lhsT = wt_sb[j][:, kk * C + ch * 128: kk * C + ch * 128 + 128]
                        for b in bg:
                            nc.tensor.matmul(
                                ps[b][:], lhsT,
                                ctx_sb[b, j][:],
                                start=(j == 0), stop=(j == 1),
                            )
                    for b in bg:
                        nc.vector.tensor_copy(tmp2[b, ch][:, kk * S:(kk + 1) * S], ps[b][:])

        def phase2(bg):
            for b in bg:
                frt = fr_pool.tile([KV + 1, S * KV], f32r, name="foldR")
                nc.sync.dma_start(frt[:], foldR_d[b])
                for sc in range(2):
                    pss = [ps2_pool.tile([128, 512], f32, name="ps2")
                           for n in range(4)]
                    for st in range(2):  # contraction chunk over i
                        lhsT = ctx_sb[b, st][:, sc * 128:(sc + 1) * 128]
                        rview = tmp2[b, st][:].rearrange("p (k z) -> p z k", k=KV)
                        for n in range(4):
                            nc.tensor.matmul(
                                pss[n][:], lhsT,
                                rview[:, n * 64:(n + 1) * 64, :],
                                start=(st == 0), stop=False,
                            )
                    lhsT3 = foldL_sb[:, b * S + sc * 128: b * S + sc * 128 + 128]
                    for n in range(4):
                        nc.tensor.matmul(
                            pss[n][:], lhsT3,
                            frt[:, n * 512:(n + 1) * 512],
                            start=False, stop=True,
                        )
                    ot = outs_pool.tile([128, S * KV], f32, name="ot")
                    for n in range(4):
                        nc.scalar.activation(ot[:, n * 512:(n + 1) * 512], pss[n][:], TANH)
                    nc.sync.dma_start(
                        out_d[b, sc * 128:(sc + 1) * 128].rearrange("s z k -> s (z k)"),
                        ot[:],
                    )

        phase1(range(0, 4))
        phase2(range(0, 4))
        phase1(range(4, 8))
        phase2(range(4, 8))

    nc.compile()
    return nc


def _install_profile_hook():
    """Register the NTFF profile hook that the image's boot skipped
    (antenv.axon_hooks shim is missing in this container)."""
    import sys as _sys
    import types as _types
    try:
        import antenv
        if "antenv.axon_hooks" not in _sys.modules:
            m = _types.ModuleType("antenv.axon_hooks")
            _h = [None]
            m.set_axon_ntff_profile_hook = lambda h: _h.__setitem__(0, h)
            m.get_axon_ntff_profile_hook = lambda: _h[0]
            _sys.modules["antenv.axon_hooks"] = m
            antenv.axon_hooks = m
        from antenv.axon_hooks import set_axon_ntff_profile_hook, get_axon_ntff_profile_hook
        if get_axon_ntff_profile_hook() is None:
            from trn_agent_boot.trn_boot import _ntff_profile_via_ctypes
            set_axon_ntff_profile_hook(_ntff_profile_via_ctypes("/opt/axon/libaxon_pjrt.so"))
    except Exception:
        pass


def run(inputs, trace=False):
    """Returns (full_output, BassKernelResults)."""
    from concourse.bass_utils import run_bass_kernel_spmd

    if trace:
        _install_profile_hook()
    per_core = _host_prep(**inputs)
    nc = _build_program()
    res = run_bass_kernel_spmd(nc, per_core, list(range(NCORES)), trace=trace)
    out = np.concatenate([res.results[c]["out"] for c in range(NCORES)], axis=3)
    return out, res


def kernel(**inputs) -> np.ndarray:
    out, _ = run(inputs, trace=False)
    return out


# revision 7
# speedup vs baseline: 1.2968x; 1.2968x over previous
"""Trainium2 Bass kernel for nn_BilinAndFwdComboVecComp.

Math (B=8, S=256, C=256, V=64):
  final[b,s,z,k] = tanh( sum_ij ctx[b,s,i] ctx[b,z,j] W'[i,j,k] + A[b,z,k] + Bt[b,s,k] )
where
  W'[i,j,k] = W[i,j,k] + (i==j) * linmul_w[k,i]          (folds the `mul` branch)
  A[b,z,k]  = ctx[b] @ (lin1_w+lindiff_w).T + (lin1_b + bias + linmul_b + lindiff_b)
  Bt[b,s,k] = ctx[b] @ (lin2_w-lindiff_w).T + lin2_b     (the `diff` branch is rank-1
                                                          per pair and merges into A/Bt)

Sharding: V split across the 8 cores (8 k-values per core). Each core:
  phase 1: tmp2_b[i,(k,z)] = sum_j Wt[j,(k,i)]^T-slices @ ctxT_b[j,z]   (W-stationary)
  phase 2: out[s,(z,k)]    = ctxT_b[:,s]^T @ tmp2_b[:,(z,k)]  (+ K=9 fold matmul that
           adds A via a ones-row and Bt via delta-rows), then tanh on ACT, DMA out.
All matmuls run as float32r (full PE rate at moving-dim >= 256, fp32-class precision).
Per-core output scratch (B,S,S,8) is concatenated on the host along k.
"""

import numpy as np

B, S, C, V = 8, 256, 256, 64
NCORES = 8
KV = V // NCORES  # k-values per core


def _host_prep(ctx, W, bias, lin1_w, lin1_b, lin2_w, lin2_b,
               linmul_w, linmul_b, lindiff_w, lindiff_b):
    f = np.float32
    ctx = np.asarray(ctx, f)
    Wp = np.array(W, f)
    Wp[np.arange(C), np.arange(C), :] += np.asarray(linmul_w, f).T
    Wt = Wp.transpose(1, 0, 2)  # [j, i, k]

    A = ctx @ (np.asarray(lin1_w, f) + np.asarray(lindiff_w, f)).T \
        + (np.asarray(lin1_b, f) + np.asarray(bias, f)
           + np.asarray(linmul_b, f) + np.asarray(lindiff_b, f))
    Bt = ctx @ (np.asarray(lin2_w, f) - np.asarray(lindiff_w, f)).T + np.asarray(lin2_b, f)

    ctxT = np.ascontiguousarray(ctx.transpose(0, 2, 1))  # [B, C, S]

    # delta in (k, z) layout: row r is 1 over the z-block of plane k==r
    delta = np.zeros((KV, KV * S), f)
    for r in range(KV):
        delta[r, r * S:(r + 1) * S] = 1.0

    per_core = []
    for c in range(NCORES):
        ks = slice(c * KV, (c + 1) * KV)
        # wt layout: [j, kk*C + i]
        wt = np.ascontiguousarray(Wt[:, :, ks].transpose(0, 2, 1).reshape(C, KV * C))
        # foldL: [9, b*S + s]; rows 0..KV-1 = Bt slice, row KV = ones
        foldL = np.empty((KV + 1, B * S), f)
        foldL[:KV] = Bt[:, :, ks].transpose(2, 0, 1).reshape(KV, B * S)
        foldL[KV] = 1.0
        # foldR: [b, 9, kk*S + z]; rows 0..KV-1 = delta, row KV = A slice (k-major)
        foldR = np.empty((B, KV + 1, KV * S), f)
        foldR[:, :KV, :] = delta[None]
        foldR[:, KV, :] = A[:, :, ks].transpose(0, 2, 1).reshape(B, KV * S)
        per_core.append({"ctxT": ctxT, "wt": wt, "foldL": foldL, "foldR": foldR})
    return per_core


def _build_program():
    import concourse.tile as tile
    import concourse.mybir as mybir
    from concourse import bacc
    from contextlib import ExitStack

    f32 = mybir.dt.float32
    f32r = mybir.dt.float32r
    TANH = mybir.ActivationFunctionType.Tanh

    nc = bacc.Bacc("TRN2", target_bir_lowering=False, debug=False)
    ctxT_d = nc.dram_tensor("ctxT", [B, C, S], f32r, kind="ExternalInput").ap()
    wt_d = nc.dram_tensor("wt", [C, KV * C], f32r, kind="ExternalInput").ap()
    foldL_d = nc.dram_tensor("foldL", [KV + 1, B * S], f32r, kind="ExternalInput").ap()
    foldR_d = nc.dram_tensor("foldR", [B, KV + 1, S * KV], f32r, kind="ExternalInput").ap()
    # out scratch is (k, z)-ordered; the host transposes back to (z, k)
    out_d = nc.dram_tensor("out", [B, S, KV, S], f32, kind="ExternalOutput").ap()

    with tile.TileContext(nc) as tc, ExitStack() as es:
        ctx_pool = es.enter_context(tc.tile_pool(name="ctxp", bufs=8))
        wt_pool = es.enter_context(tc.tile_pool(name="wtp", bufs=2))
        fl_pool = es.enter_context(tc.tile_pool(name="flp", bufs=1))
        fr_pool = es.enter_context(tc.tile_pool(name="frp", bufs=2))
        tmp2_pool = es.enter_context(tc.tile_pool(name="tmp2p", bufs=12))
        outs_pool = es.enter_context(tc.tile_pool(name="outsp", bufs=3))
        ps1_pool = es.enter_context(tc.tile_pool(name="ps1", bufs=4, space="PSUM"))
        ps2_pool = es.enter_context(tc.tile_pool(name="ps2", bufs=4, space="PSUM"))

        # ctx pair tiles: [128, 512] = ctxT[2p, jchunk] | ctxT[2p+1, jchunk]
        ctxp_sb = {}
        for p in range(B // 2):
            for j in range(2):
                t = ctx_pool.tile([128, 2 * S], f32r, name=f"ctx_{p}_{j}", bufs=1)
                nc.sync.dma_start(t[:, 0:S], ctxT_d[2 * p, j * 128:(j + 1) * 128, :])
                nc.sync.dma_start(t[:, S:2 * S], ctxT_d[2 * p + 1, j * 128:(j + 1) * 128, :])
                ctxp_sb[p, j] = t
        wt_sb = []
        for j in range(2):
            t = wt_pool.tile([128, KV * C], f32r, name=f"wt_{j}", bufs=1)
            nc.sync.dma_start(t[:], wt_d[j * 128:(j + 1) * 128, :])
            wt_sb.append(t)
        foldL_sb = fl_pool.tile([KV + 1, B * S], f32r, name="foldL", bufs=1)
        nc.sync.dma_start(foldL_sb[:], foldL_d[:])

        tmp2 = {}
        copy_parity = [0]

        def phase1(pg):  # pg: pair indices
            for ch in range(2):  # i-chunk (output partition of tmp2)
                for p in pg:
                    tmp2[2 * p, ch] = tmp2_pool.tile([128, KV * S], f32r, name="tmp2")
                    tmp2[2 * p + 1, ch] = tmp2_pool.tile([128, KV * S], f32r, name="tmp2")
                for kk in range(KV):
                    ps = {}
                    for p in pg:
                        ps[p] = ps1_pool.tile([128, 2 * S], f32, name="ps1")
                    for j in range(2):  # contraction chunk
                        lhsT = wt_sb[j][:, kk * C + ch * 128: kk * C + ch * 128 + 128]
                        for p in pg:
                            nc.tensor.matmul(
                                ps[p][:], lhsT, ctxp_sb[p, j][:],
                                start=(j == 0), stop=(j == 1),
                            )
                    for p in pg:
                        for h in range(2):
                            dst = tmp2[2 * p + h, ch][:, kk * S:(kk + 1) * S]
                            src = ps[p][:, h * S:(h + 1) * S]
                            if copy_parity[0] % 2 == 0:
                                nc.vector.tensor_copy(dst, src)
                            else:
                                nc.scalar.copy(dst, src)
                            copy_parity[0] += 1

        def phase2(bg):
            for b in bg:
                frt = fr_pool.tile([KV + 1, S * KV], f32r, name="foldR")
                nc.sync.dma_start(frt[:], foldR_d[b])
                for sc in range(2):
                    pss = [ps2_pool.tile([128, 512], f32, name="ps2")
                           for n in range(4)]
                    for st in range(2):  # contraction chunk over i
                        lhsT = ctxp_sb[b // 2, st][:, (b % 2) * S + sc * 128:
                                                   (b % 2) * S + sc * 128 + 128]
                        for n in range(4):  # k-pair planes, contiguous rhs
                            nc.tensor.matmul(
                                pss[n][:], lhsT,
                                tmp2[b, st][:, n * 512:(n + 1) * 512],
                                start=(st == 0), stop=False,
                            )
                    lhsT3 = foldL_sb[:, b * S + sc * 128: b * S + sc * 128 + 128]
                    for n in range(4):
                        nc.tensor.matmul(
                            pss[n][:], lhsT3,
                            frt[:, n * 512:(n + 1) * 512],
                            start=False, stop=True,
                        )
                    ot = outs_pool.tile([128, S * KV], f32, name="ot")
                    for n in range(4):
                        nc.scalar.activation(ot[:, n * 512:(n + 1) * 512], pss[n][:], TANH)
                    nc.sync.dma_start(
                        out_d[b, sc * 128:(sc + 1) * 128].rearrange("s k z -> s (k z)"),
                        ot[:],
                    )

        phase1([0, 1])
        phase2(range(0, 4))
        phase1([2, 3])
        phase2(range(4, 8))

    nc.compile()
    return nc


def _install_profile_hook():
    """Register the NTFF profile hook that the image's boot skipped
    (antenv.axon_hooks shim is missing in this container)."""
    import sys as _sys
    import types as _types
    try:
        import antenv
        if "antenv.axon_hooks" not in _sys.modules:
            m = _types.ModuleType("antenv.axon_hooks")
            _h = [None]
            m.set_axon_ntff_profile_hook = lambda h: _h.__setitem__(0, h)
            m.get_axon_ntff_profile_hook = lambda: _h[0]
            _sys.modules["antenv.axon_hooks"] = m
            antenv.axon_hooks = m
        from antenv.axon_hooks import set_axon_ntff_profile_hook, get_axon_ntff_profile_hook
        if get_axon_ntff_profile_hook() is None:
            from trn_agent_boot.trn_boot import _ntff_profile_via_ctypes
            set_axon_ntff_profile_hook(_ntff_profile_via_ctypes("/opt/axon/libaxon_pjrt.so"))
    except Exception:
        pass


def run(inputs, trace=False):
    """Returns (full_output, BassKernelResults)."""
    from concourse.bass_utils import run_bass_kernel_spmd

    if trace:
        _install_profile_hook()
    per_core = _host_prep(**inputs)
    nc = _build_program()
    res = run_bass_kernel_spmd(nc, per_core, list(range(NCORES)), trace=trace)
    # per-core scratch is (B, S, KV, S) with k-major planes: swap to (B,S,S,KV)
    out = np.concatenate(
        [res.results[c]["out"].transpose(0, 1, 3, 2) for c in range(NCORES)], axis=3)
    out = np.ascontiguousarray(out)
    return out, res


def kernel(**inputs) -> np.ndarray:
    out, _ = run(inputs, trace=False)
    return out


# revision 11
# speedup vs baseline: 1.3692x; 1.0558x over previous
"""Trainium2 Bass kernel for nn_BilinAndFwdComboVecComp.

Math (B=8, S=256, C=256, V=64):
  final[b,s,z,k] = tanh( sum_ij ctx[b,s,i] ctx[b,z,j] W'[i,j,k] + A[b,z,k] + Bt[b,s,k] )
where
  W'[i,j,k] = W[i,j,k] + (i==j) * linmul_w[k,i]          (folds the `mul` branch)
  A[b,z,k]  = ctx[b] @ (lin1_w+lindiff_w).T + (lin1_b + bias + linmul_b + lindiff_b)
  Bt[b,s,k] = ctx[b] @ (lin2_w-lindiff_w).T + lin2_b     (the `diff` branch is rank-1
                                                          per pair and merges into A/Bt)

Sharding: V split across the 8 cores (8 k-values per core). Each core:
  phase 1: tmp2_b[i,(k,z)] = sum_j Wt[j,(k,i)]^T-slices @ ctxT_b[j,z]   (W-stationary)
  phase 2: out[s,(z,k)]    = ctxT_b[:,s]^T @ tmp2_b[:,(z,k)]  (+ K=9 fold matmul that
           adds A via a ones-row and Bt via delta-rows), then tanh on ACT, DMA out.
All matmuls run as float32r (full PE rate at moving-dim >= 256, fp32-class precision).
Per-core output scratch (B,S,S,8) is concatenated on the host along k.
"""

import numpy as np

B, S, C, V = 8, 256, 256, 64
NCORES = 8
KV = V // NCORES  # k-values per core


def _host_prep(ctx, W, bias, lin1_w, lin1_b, lin2_w, lin2_b,
               linmul_w, linmul_b, lindiff_w, lindiff_b):
    f = np.float32
    ctx = np.asarray(ctx, f)
    Wp = np.array(W, f)
    Wp[np.arange(C), np.arange(C), :] += np.asarray(linmul_w, f).T
    Wt = Wp.transpose(1, 0, 2)  # [j, i, k]

    A = ctx @ (np.asarray(lin1_w, f) + np.asarray(lindiff_w, f)).T \
        + (np.asarray(lin1_b, f) + np.asarray(bias, f)
           + np.asarray(linmul_b, f) + np.asarray(lindiff_b, f))
    Bt = ctx @ (np.asarray(lin2_w, f) - np.asarray(lindiff_w, f)).T + np.asarray(lin2_b, f)

    ctxT = np.ascontiguousarray(ctx.transpose(0, 2, 1))  # [B, C, S]

    # delta in (k, z) layout: row r is 1 over the z-block of plane k==r
    delta = np.zeros((KV, KV * S), f)
    for r in range(KV):
        delta[r, r * S:(r + 1) * S] = 1.0

    per_core = []
    for c in range(NCORES):
        ks = slice(c * KV, (c + 1) * KV)
        # wt layout: [j, kk*C + i]
        wt = np.ascontiguousarray(Wt[:, :, ks].transpose(0, 2, 1).reshape(C, KV * C))
        # foldL: [9, b*S + s]; rows 0..KV-1 = Bt slice, row KV = ones
        foldL = np.empty((KV + 1, B * S), f)
        foldL[:KV] = Bt[:, :, ks].transpose(2, 0, 1).reshape(KV, B * S)
        foldL[KV] = 1.0
        # foldR: [b, 9, kk*S + z]; rows 0..KV-1 = delta, row KV = A slice (k-major)
        foldR = np.empty((B, KV + 1, KV * S), f)
        foldR[:, :KV, :] = delta[None]
        foldR[:, KV, :] = A[:, :, ks].transpose(0, 2, 1).reshape(B, KV * S)
        per_core.append({"ctxT": ctxT, "wt": wt, "foldL": foldL, "foldR": foldR})
    return per_core


def _build_program():
    import concourse.tile as tile
    import concourse.mybir as mybir
    from concourse import bacc
    from contextlib import ExitStack

    f32 = mybir.dt.float32
    f32r = mybir.dt.float32r
    TANH = mybir.ActivationFunctionType.Tanh

    nc = bacc.Bacc("TRN2", target_bir_lowering=False, debug=False)
    ctxT_d = nc.dram_tensor("ctxT", [B, C, S], f32r, kind="ExternalInput").ap()
    wt_d = nc.dram_tensor("wt", [C, KV * C], f32r, kind="ExternalInput").ap()
    foldL_d = nc.dram_tensor("foldL", [KV + 1, B * S], f32r, kind="ExternalInput").ap()
    foldR_d = nc.dram_tensor("foldR", [B, KV + 1, S * KV], f32r, kind="ExternalInput").ap()
    # out scratch is (k, z)-ordered; the host transposes back to (z, k)
    out_d = nc.dram_tensor("out", [B, S, KV, S], f32, kind="ExternalOutput").ap()

    with tile.TileContext(nc) as tc, ExitStack() as es:
        ctx_pool = es.enter_context(tc.tile_pool(name="ctxp", bufs=8))
        wt_pool = es.enter_context(tc.tile_pool(name="wtp", bufs=2))
        fl_pool = es.enter_context(tc.tile_pool(name="flp", bufs=1))
        fr_pool = es.enter_context(tc.tile_pool(name="frp", bufs=2))
        tmp2_pool = es.enter_context(tc.tile_pool(name="tmp2p", bufs=12))
        outs_pool = es.enter_context(tc.tile_pool(name="outsp", bufs=3))
        ps1_pool = es.enter_context(tc.tile_pool(name="ps1", bufs=4, space="PSUM"))
        ps2_pool = es.enter_context(tc.tile_pool(name="ps2", bufs=4, space="PSUM"))

        # ctx pair tiles: [128, 512] = ctxT[2p, jchunk] | ctxT[2p+1, jchunk]
        ctxp_sb = {}
        for p in range(B // 2):
            for j in range(2):
                t = ctx_pool.tile([128, 2 * S], f32r, name=f"ctx_{p}_{j}", bufs=1)
                nc.sync.dma_start(t[:, 0:S], ctxT_d[2 * p, j * 128:(j + 1) * 128, :])
                nc.sync.dma_start(t[:, S:2 * S], ctxT_d[2 * p + 1, j * 128:(j + 1) * 128, :])
                ctxp_sb[p, j] = t
        wt_sb = []
        for j in range(2):
            t = wt_pool.tile([128, KV * C], f32r, name=f"wt_{j}", bufs=1)
            nc.sync.dma_start(t[:], wt_d[j * 128:(j + 1) * 128, :])
            wt_sb.append(t)
        foldL_sb = fl_pool.tile([KV + 1, B * S], f32r, name="foldL", bufs=1)
        nc.sync.dma_start(foldL_sb[:], foldL_d[:])

        tmp2 = {}

        def phase1(pg):  # pg: pair indices
            for ch in range(2):  # i-chunk (output partition of tmp2)
                for p in pg:
                    tmp2[2 * p, ch] = tmp2_pool.tile([128, KV * S], f32r, name="tmp2")
                    tmp2[2 * p + 1, ch] = tmp2_pool.tile([128, KV * S], f32r, name="tmp2")
                for kk in range(KV):
                    ps = {}
                    for p in pg:
                        ps[p] = ps1_pool.tile([128, 2 * S], f32, name="ps1")
                    for j in range(2):  # contraction chunk
                        lhsT = wt_sb[j][:, kk * C + ch * 128: kk * C + ch * 128 + 128]
                        for p in pg:
                            nc.tensor.matmul(
                                ps[p][:], lhsT, ctxp_sb[p, j][:],
                                start=(j == 0), stop=(j == 1),
                            )
                    for p in pg:
                        for h in range(2):
                            dst = tmp2[2 * p + h, ch][:, kk * S:(kk + 1) * S]
                            src = ps[p][:, h * S:(h + 1) * S]
                            nc.vector.tensor_copy(dst, src)

        def phase2(bg):
            for b in bg:
                frt = fr_pool.tile([KV + 1, S * KV], f32r, name="foldR")
                nc.sync.dma_start(frt[:], foldR_d[b])
                for sc in range(2):
                    pss = [ps2_pool.tile([128, 512], f32, name="ps2")
                           for n in range(4)]
                    for st in range(2):  # contraction chunk over i
                        lhsT = ctxp_sb[b // 2, st][:, (b % 2) * S + sc * 128:
                                                   (b % 2) * S + sc * 128 + 128]
                        for n in range(4):  # k-pair planes, contiguous rhs
                            nc.tensor.matmul(
                                pss[n][:], lhsT,
                                tmp2[b, st][:, n * 512:(n + 1) * 512],
                                start=(st == 0), stop=False,
                            )
                    lhsT3 = foldL_sb[:, b * S + sc * 128: b * S + sc * 128 + 128]
                    for n in range(4):
                        nc.tensor.matmul(
                            pss[n][:], lhsT3,
                            frt[:, n * 512:(n + 1) * 512],
                            start=False, stop=True,
                        )
                    ot = outs_pool.tile([128, S * KV], f32, name="ot")
                    for n in range(4):
                        nc.scalar.activation(ot[:, n * 512:(n + 1) * 512], pss[n][:], TANH)
                    nc.sync.dma_start(
                        out_d[b, sc * 128:(sc + 1) * 128].rearrange("s k z -> s (k z)"),
                        ot[:],
                    )

        # fine-grained interleave: PE always has phase-1 fill-in work while
        # phase-2 waits on PSUM drains
        phase1([0])
        phase1([1])
        phase2([0, 1])
        phase1([2])
        phase2([2, 3])
        phase1([3])
        phase2([4, 5])
        phase2([6, 7])

    nc.compile()
    return nc


def _install_profile_hook():
    """Register the NTFF profile hook that the image's boot skipped
    (antenv.axon_hooks shim is missing in this container)."""
    import sys as _sys
    import types as _types
    try:
        import antenv
        if "antenv.axon_hooks" not in _sys.modules:
            m = _types.ModuleType("antenv.axon_hooks")
            _h = [None]
            m.set_axon_ntff_profile_hook = lambda h: _h.__setitem__(0, h)
            m.get_axon_ntff_profile_hook = lambda: _h[0]
            _sys.modules["antenv.axon_hooks"] = m
            antenv.axon_hooks = m
        from antenv.axon_hooks import set_axon_ntff_profile_hook, get_axon_ntff_profile_hook
        if get_axon_ntff_profile_hook() is None:
            from trn_agent_boot.trn_boot import _ntff_profile_via_ctypes
            set_axon_ntff_profile_hook(_ntff_profile_via_ctypes("/opt/axon/libaxon_pjrt.so"))
    except Exception:
        pass


def _patch_walrus_ldw_opt():
    """Enable walrus LDWEIGHTS dedup (concourse hardcodes it off). With fp32r
    matmuls walrus emits one LDWEIGHTS per matmul; repeated identical loads
    serialize against the matmul stream (same-row-group loads can't pull
    ahead), costing ~170-300ns per matmul."""
    import os
    if os.environ.get("KERNEL_LDW_OPT", "1") != "1":
        return
    import concourse.bass_utils as bu
    if getattr(bu.run_command, "_ldw_patched", False):
        return
    orig = bu.run_command

    def patched(argv, **kw):
        argv = ["--enable-ldw-opt=true" if a == "--enable-ldw-opt=false" else a
                for a in argv]
        return orig(argv, **kw)

    patched._ldw_patched = True
    bu.run_command = patched


def run(inputs, trace=False):
    """Returns (full_output, BassKernelResults)."""
    from concourse.bass_utils import run_bass_kernel_spmd

    _patch_walrus_ldw_opt()
    if trace:
        _install_profile_hook()
    per_core = _host_prep(**inputs)
    nc = _build_program()
    res = run_bass_kernel_spmd(nc, per_core, list(range(NCORES)), trace=trace)
    # per-core scratch is (B, S, KV, S) with k-major planes: swap to (B,S,S,KV)
    out = np.concatenate(
        [res.results[c]["out"].transpose(0, 1, 3, 2) for c in range(NCORES)], axis=3)
    out = np.ascontiguousarray(out)
    return out, res


def kernel(**inputs) -> np.ndarray:
    out, _ = run(inputs, trace=False)
    return out


# revision 12
# speedup vs baseline: 1.4670x; 1.0714x over previous
"""Trainium2 Bass kernel for nn_BilinAndFwdComboVecComp.

Math (B=8, S=256, C=256, V=64):
  final[b,s,z,k] = tanh( sum_ij ctx[b,s,i] ctx[b,z,j] W'[i,j,k] + A[b,z,k] + Bt[b,s,k] )
where
  W'[i,j,k] = W[i,j,k] + (i==j) * linmul_w[k,i]          (folds the `mul` branch)
  A[b,z,k]  = ctx[b] @ (lin1_w+lindiff_w).T + (lin1_b + bias + linmul_b + lindiff_b)
  Bt[b,s,k] = ctx[b] @ (lin2_w-lindiff_w).T + lin2_b     (the `diff` branch is rank-1
                                                          per pair and merges into A/Bt)

Sharding: V split across the 8 cores (8 k-values per core). Each core:
  phase 1: tmp2_b[i,(k,z)] = sum_j Wt[j,(k,i)]^T-slices @ ctxT_b[j,z]   (W-stationary)
  phase 2: out[s,(z,k)]    = ctxT_b[:,s]^T @ tmp2_b[:,(z,k)]  (+ K=9 fold matmul that
           adds A via a ones-row and Bt via delta-rows), then tanh on ACT, DMA out.
All matmuls run as float32r (full PE rate at moving-dim >= 256, fp32-class precision).
Per-core output scratch (B,S,S,8) is concatenated on the host along k.
"""

import numpy as np

B, S, C, V = 8, 256, 256, 64
NCORES = 8
KV = V // NCORES  # k-values per core


def _host_prep(ctx, W, bias, lin1_w, lin1_b, lin2_w, lin2_b,
               linmul_w, linmul_b, lindiff_w, lindiff_b):
    f = np.float32
    ctx = np.asarray(ctx, f)
    Wp = np.array(W, f)
    Wp[np.arange(C), np.arange(C), :] += np.asarray(linmul_w, f).T
    Wt = Wp.transpose(1, 0, 2)  # [j, i, k]

    A = ctx @ (np.asarray(lin1_w, f) + np.asarray(lindiff_w, f)).T \
        + (np.asarray(lin1_b, f) + np.asarray(bias, f)
           + np.asarray(linmul_b, f) + np.asarray(lindiff_b, f))
    Bt = ctx @ (np.asarray(lin2_w, f) - np.asarray(lindiff_w, f)).T + np.asarray(lin2_b, f)

    ctxT = np.ascontiguousarray(ctx.transpose(0, 2, 1))  # [B, C, S]

    # delta in (k, z) layout: row r is 1 over the z-block of plane k==r
    delta = np.zeros((KV, KV * S), f)
    for r in range(KV):
        delta[r, r * S:(r + 1) * S] = 1.0

    per_core = []
    for c in range(NCORES):
        ks = slice(c * KV, (c + 1) * KV)
        # wt layout: [j, kk*C + i]
        wt = np.ascontiguousarray(Wt[:, :, ks].transpose(0, 2, 1).reshape(C, KV * C))
        # foldL: [9, b*S + s]; rows 0..KV-1 = Bt slice, row KV = ones
        foldL = np.empty((KV + 1, B * S), f)
        foldL[:KV] = Bt[:, :, ks].transpose(2, 0, 1).reshape(KV, B * S)
        foldL[KV] = 1.0
        # foldR: [b, 9, kk*S + z]; rows 0..KV-1 = delta, row KV = A slice (k-major)
        foldR = np.empty((B, KV + 1, KV * S), f)
        foldR[:, :KV, :] = delta[None]
        foldR[:, KV, :] = A[:, :, ks].transpose(0, 2, 1).reshape(B, KV * S)
        per_core.append({"ctxT": ctxT, "wt": wt, "foldL": foldL, "foldR": foldR})
    return per_core


def _build_program():
    import concourse.tile as tile
    import concourse.mybir as mybir
    from concourse import bacc
    from contextlib import ExitStack

    f32 = mybir.dt.float32
    f32r = mybir.dt.float32r
    TANH = mybir.ActivationFunctionType.Tanh

    nc = bacc.Bacc("TRN2", target_bir_lowering=False, debug=False)
    ctxT_d = nc.dram_tensor("ctxT", [B, C, S], f32r, kind="ExternalInput").ap()
    wt_d = nc.dram_tensor("wt", [C, KV * C], f32r, kind="ExternalInput").ap()
    foldL_d = nc.dram_tensor("foldL", [KV + 1, B * S], f32r, kind="ExternalInput").ap()
    foldR_d = nc.dram_tensor("foldR", [B, KV + 1, S * KV], f32r, kind="ExternalInput").ap()
    # out scratch is (k, z)-ordered; the host transposes back to (z, k)
    out_d = nc.dram_tensor("out", [B, S, KV, S], f32, kind="ExternalOutput").ap()

    with tile.TileContext(nc) as tc, ExitStack() as es:
        ctx_pool = es.enter_context(tc.tile_pool(name="ctxp", bufs=8))
        wt_pool = es.enter_context(tc.tile_pool(name="wtp", bufs=2))
        fl_pool = es.enter_context(tc.tile_pool(name="flp", bufs=1))
        fr_pool = es.enter_context(tc.tile_pool(name="frp", bufs=2))
        tmp2_pool = es.enter_context(tc.tile_pool(name="tmp2p", bufs=6))
        outs_pool = es.enter_context(tc.tile_pool(name="outsp", bufs=2))
        ps1_pool = es.enter_context(tc.tile_pool(name="ps1", bufs=4, space="PSUM"))
        ps2_pool = es.enter_context(tc.tile_pool(name="ps2", bufs=4, space="PSUM"))

        # ctx pair tiles: [128, 512] = ctxT[2p, jchunk] | ctxT[2p+1, jchunk]
        ctxp_sb = {}
        for p in range(B // 2):
            for j in range(2):
                t = ctx_pool.tile([128, 2 * S], f32r, name=f"ctx_{p}_{j}", bufs=1)
                nc.sync.dma_start(t[:, 0:S], ctxT_d[2 * p, j * 128:(j + 1) * 128, :])
                nc.sync.dma_start(t[:, S:2 * S], ctxT_d[2 * p + 1, j * 128:(j + 1) * 128, :])
                ctxp_sb[p, j] = t
        wt_sb = []
        for j in range(2):
            t = wt_pool.tile([128, KV * C], f32r, name=f"wt_{j}", bufs=1)
            nc.sync.dma_start(t[:], wt_d[j * 128:(j + 1) * 128, :])
            wt_sb.append(t)
        foldL_sb = fl_pool.tile([KV + 1, B * S], f32r, name="foldL", bufs=1)
        nc.sync.dma_start(foldL_sb[:], foldL_d[:])

        tmp2p = {}

        def phase1(pg):  # pg: pair indices
            for ch in range(2):  # i-chunk (output partition of tmp2)
                for p in pg:
                    # pair tile, layout (h=b-half, k, z)
                    tmp2p[p, ch] = tmp2_pool.tile([128, 2 * KV * S], f32r, name="tmp2")
                for kk in range(KV):
                    ps = {}
                    for p in pg:
                        ps[p] = ps1_pool.tile([128, 2 * S], f32, name="ps1")
                    for j in range(2):  # contraction chunk
                        lhsT = wt_sb[j][:, kk * C + ch * 128: kk * C + ch * 128 + 128]
                        for p in pg:
                            nc.tensor.matmul(
                                ps[p][:], lhsT, ctxp_sb[p, j][:],
                                start=(j == 0), stop=(j == 1),
                            )
                    for p in pg:
                        # one copy per bank: psum (h, z) -> pair tile (h, kk, z)
                        dst = tmp2p[p, ch][:].rearrange("q (h k z) -> q h k z", h=2, k=KV)
                        nc.vector.tensor_copy(dst[:, :, kk, :], ps[p][:].rearrange(
                            "q (h z) -> q h z", h=2))

        def phase2(bg):
            for b in bg:
                frt = fr_pool.tile([KV + 1, S * KV], f32r, name="foldR")
                nc.sync.dma_start(frt[:], foldR_d[b])
                for sc in range(2):
                    pss = [ps2_pool.tile([128, 512], f32, name="ps2")
                           for n in range(4)]
                    for st in range(2):  # contraction chunk over i
                        lhsT = ctxp_sb[b // 2, st][:, (b % 2) * S + sc * 128:
                                                   (b % 2) * S + sc * 128 + 128]
                        hoff = (b % 2) * KV * S
                        for n in range(4):  # k-pair planes, contiguous rhs
                            nc.tensor.matmul(
                                pss[n][:], lhsT,
                                tmp2p[b // 2, st][:, hoff + n * 512:hoff + (n + 1) * 512],
                                start=(st == 0), stop=False,
                            )
                    lhsT3 = foldL_sb[:, b * S + sc * 128: b * S + sc * 128 + 128]
                    for n in range(4):
                        nc.tensor.matmul(
                            pss[n][:], lhsT3,
                            frt[:, n * 512:(n + 1) * 512],
                            start=False, stop=True,
                        )
                    ot = outs_pool.tile([128, S * KV], f32, name="ot")
                    for n in range(4):
                        nc.scalar.activation(ot[:, n * 512:(n + 1) * 512], pss[n][:], TANH)
                    nc.sync.dma_start(
                        out_d[b, sc * 128:(sc + 1) * 128].rearrange("s k z -> s (k z)"),
                        ot[:],
                    )

        # 2-pair phase-1 groups amortize weight loads; early phase-2 work is
        # PE fill-in while phase-1 waits on DVE drains (and vice versa later)
        phase1([0, 1])
        phase2([0])
        phase2([1])
        phase1([2, 3])
        for b in range(2, 8):
            phase2([b])

    nc.compile()
    return nc


def _install_profile_hook():
    """Register the NTFF profile hook that the image's boot skipped
    (antenv.axon_hooks shim is missing in this container)."""
    import sys as _sys
    import types as _types
    try:
        import antenv
        if "antenv.axon_hooks" not in _sys.modules:
            m = _types.ModuleType("antenv.axon_hooks")
            _h = [None]
            m.set_axon_ntff_profile_hook = lambda h: _h.__setitem__(0, h)
            m.get_axon_ntff_profile_hook = lambda: _h[0]
            _sys.modules["antenv.axon_hooks"] = m
            antenv.axon_hooks = m
        from antenv.axon_hooks import set_axon_ntff_profile_hook, get_axon_ntff_profile_hook
        if get_axon_ntff_profile_hook() is None:
            from trn_agent_boot.trn_boot import _ntff_profile_via_ctypes
            set_axon_ntff_profile_hook(_ntff_profile_via_ctypes("/opt/axon/libaxon_pjrt.so"))
    except Exception:
        pass


def _patch_walrus_ldw_opt():
    """Enable walrus LDWEIGHTS dedup (concourse hardcodes it off). With fp32r
    matmuls walrus emits one LDWEIGHTS per matmul; repeated identical loads
    serialize against the matmul stream (same-row-group loads can't pull
    ahead), costing ~170-300ns per matmul."""
    import os
    if os.environ.get("KERNEL_LDW_OPT", "1") != "1":
        return
    import concourse.bass_utils as bu
    if getattr(bu.run_command, "_ldw_patched", False):
        return
    orig = bu.run_command

    def patched(argv, **kw):
        argv = ["--enable-ldw-opt=true" if a == "--enable-ldw-opt=false" else a
                for a in argv]
        return orig(argv, **kw)

    patched._ldw_patched = True
    bu.run_command = patched


def run(inputs, trace=False):
    """Returns (full_output, BassKernelResults)."""
    from concourse.bass_utils import run_bass_kernel_spmd

    _patch_walrus_ldw_opt()
    if trace:
        _install_profile_hook()
    per_core = _host_prep(**inputs)
    nc = _build_program()
    res = run_bass_kernel_spmd(nc, per_core, list(range(NCORES)), trace=trace)
    # per-core scratch is (B, S, KV, S) with k-major planes: swap to (B,S,S,KV)
    out = np.concatenate(
        [res.results[c]["out"].transpose(0, 1, 3, 2) for c in range(NCORES)], axis=3)
    out = np.ascontiguousarray(out)
    return out, res


def kernel(**inputs) -> np.ndarray:
    out, _ = run(inputs, trace=False)
    return out
